# revision 26
# baseline (speedup 1.0000x reference)
"""Multi-head attention (B=2, T=2048, D=1024, H=16, d_k=64) on 8 trn2 cores.

Sharding: tensor-parallel over heads (4 TP groups of 4 heads) x data-parallel
over batch (2). Core c handles batch (c // 4) and heads [4*(c%4), 4*(c%4)+4).

Per-core device program (all matmuls bf16 inputs, fp32 PSUM accumulate):
  QT/KT = W @ X^T            [256, 2048] (d-major layout, + bias via DVE)
  V     = X @ Wv^T           [2048, 256] (s-major, ones column appended)
  ST    = K^T-slices^T Q^T   scores transposed [s, t]; softmax runs over the
                             partition dim implicitly:
  P     = exp(ST / 32)       (ACT, no max-subtraction: |ST/32| < ~1)
  O'/r  = [V|1]^T P          one matmul gives both the unnormalized output and
                             the softmax denominator r (row 64)
  O     = O' * (1/r)         (DVE, gpsimd partition_broadcast for 1/r)
  Y     = O^T @ Wo-slice     [2048, 1024] partial, summed across TP on host.

Host folds the V bias and output bias: attn rows sum to 1, so
out += bo + bv @ Wo^T once per batch after the TP reduction.
"""

import numpy as np
import ml_dtypes

import concourse.bass as bass
from concourse import bacc
import concourse.mybir as mybir
import concourse.tile as tile
from concourse.bass_utils import run_bass_kernel_spmd

# ---------------------------------------------------------------------------

P = 128
T = 2048          # sequence length
DM = 1024         # d_model
DC = 256          # per-core head dims (4 heads x 64)
NH = 4            # heads per core
DK = 64
ET = DM // P      # 8 contraction tiles
TT = T // P       # 16 s/t tiles
N_CORES = 8
BF16 = mybir.dt.bfloat16
F32 = mybir.dt.float32
F32R = mybir.dt.float32r
SCALE = 1.0 / 32.0  # 1/sqrt(d_model)

_ts = bass.ts


def build_nc():
    nc = bacc.Bacc("TRN2", target_bir_lowering=False, debug=False)
    xT = nc.dram_tensor("xT", (DM, T), BF16, kind="ExternalInput")
    wq = nc.dram_tensor("wq", (DM, DC), BF16, kind="ExternalInput")
    wk = nc.dram_tensor("wk", (DM, DC), BF16, kind="ExternalInput")
    wv = nc.dram_tensor("wv", (DM, DC), BF16, kind="ExternalInput")
    wo = nc.dram_tensor("wo", (DC, DM), BF16, kind="ExternalInput")
    bqk = nc.dram_tensor("bqk", (P, 4), F32, kind="ExternalInput")
    onec = nc.dram_tensor("onec", (P, TT * NH), BF16, kind="ExternalInput")
    onesr = nc.dram_tensor("onesr", (1, DK), F32R, kind="ExternalInput")
    y = nc.dram_tensor("y", (2, T, DM), F32, kind="ExternalOutput")

    with tile.TileContext(nc) as tc:
        from contextlib import ExitStack
        with ExitStack() as ctx:
            cst = ctx.enter_context(tc.tile_pool(name="cst", bufs=1))
            wq_sb = cst.tile([P, ET, DC], BF16, tag="wq")
            wk_sb = cst.tile([P, ET, DC], BF16, tag="wk")
            wv_sb = cst.tile([P, ET, DC], BF16, tag="wv")
            wo_sb = cst.tile([P, 2, DM], BF16, tag="wo")
            bqk_sb = cst.tile([P, 4], F32, tag="bqk")
            ones_r = cst.tile([1, DK], F32R, tag="ones_r")
            xt_sb = cst.tile([P, ET, T], BF16, tag="xt")
            qt_sb = cst.tile([P, 2, T], BF16, tag="qt")
            kt_sb = cst.tile([P, 2, T], BF16, tag="kt")
            v_sb = cst.tile([P, TT, NH, DK + 1], BF16, tag="v")
            ot_sb = cst.tile([P, 2, T], BF16, tag="ot")

            nc.sync.dma_start(wq_sb[:], wq[:].rearrange("(e p) d -> p e d", p=P))
            nc.sync.dma_start(wk_sb[:], wk[:].rearrange("(e p) d -> p e d", p=P))
            nc.sync.dma_start(bqk_sb[:], bqk[:])
            xr = xT[:].rearrange("(e p) t -> p e t", p=P)
            for e in range(ET):
                nc.sync.dma_start(xt_sb[:, e, :], xr[:, e, :])
            nc.sync.dma_start(wv_sb[:], wv[:].rearrange("(e p) d -> p e d", p=P))
            nc.sync.dma_start(wo_sb[:], wo[:].rearrange("(g p) f -> p g f", p=P))
            nc.sync.dma_start(ones_r[:], onesr[:])
            nc.sync.dma_start(
                v_sb[:, :, :, DK:DK + 1],
                onec[:].rearrange("p (s h) -> p s h", s=TT, h=NH).unsqueeze(-1))

            ps_st = ctx.enter_context(
                tc.tile_pool(name="ps_st", bufs=2, space="PSUM"))
            ps_av = ctx.enter_context(
                tc.tile_pool(name="ps_av", bufs=4, space="PSUM"))
            ptp = ctx.enter_context(tc.tile_pool(name="ptp", bufs=24))
            rsp = ctx.enter_context(tc.tile_pool(name="rsp", bufs=2))
            o64 = ctx.enter_context(tc.tile_pool(name="o64", bufs=3))
            ysp = ctx.enter_context(tc.tile_pool(name="ysp", bufs=3))

            # ---- QT / KT dt=0 projections (pair-0 inputs) ------------------
            for w_sb, dst, bcol in ((wq_sb, qt_sb, 0), (wk_sb, kt_sb, 2)):
                for tcp in range(2):
                    ps = ps_st.tile([P, 1024], F32, tag="ps", name="ps")
                    for u in range(2):
                        t0 = tcp * 1024 + u * 512
                        for e in range(ET):
                            nc.tensor.matmul(
                                ps[:, u * 512:(u + 1) * 512],
                                w_sb[:, e, 0:P],
                                xt_sb[:, e, t0:t0 + 512],
                                start=(e == 0), stop=(e == ET - 1))
                    nc.vector.tensor_scalar_add(
                        dst[:, 0, tcp * 1024:(tcp + 1) * 1024], ps[:],
                        bqk_sb[:, bcol:bcol + 1])

            # dt=1 projection work, fed into head 0's stream as PE filler
            proj1 = [(w_sb, dst, bcol, t0)
                     for w_sb, dst, bcol in ((wq_sb, qt_sb, 0), (wk_sb, kt_sb, 2))
                     for t0 in (0, 512, 1024, 1536)]

            def av_step(prev_state, st):
                hp_p, h2_p, pt_p, av_p = prev_state
                hcol = hp_p * 2 + h2_p
                for tc4 in range(4):
                    nc.tensor.matmul(
                        av_p[tc4][0:DK + 1, :],
                        v_sb[:, st, hcol, :],
                        pt_p[st][:, tc4 * 512:(tc4 + 1) * 512],
                        start=(st == 0), stop=(st == TT - 1))

            def av_drain(prev_state):
                hp_p, h2_p, pt_p, av_p = prev_state
                for tc4 in range(4):
                    av = av_p[tc4]
                    oa = o64.tile([DK, 512], BF16, tag="oa", name="oa")
                    nc.vector.tensor_copy(oa[:], av[0:DK, :])
                    rs = rsp.tile([DK + 1, 512], F32, tag="rs", name="rs")
                    nc.vector.tensor_copy(rs[DK:DK + 1, :], av[DK:DK + 1, :])
                    rs0 = rsp.tile([1, 512], F32, tag="rs0", name="rs0")
                    nc.sync.dma_start(rs0[0:1, :], rs[DK:DK + 1, :])
                    rf = rsp.tile([1, 512], F32, tag="rf", name="rf")
                    nc.vector.reciprocal_approx_fast(rf[:], rs0[:])
                    rr = rsp.tile([1, 512], F32R, tag="rr", name="rr")
                    nc.vector.tensor_copy(rr[:], rf[:])
                    bc = ps_av.tile([P, 512], F32, tag="ps512", name="ps512")
                    nc.tensor.matmul(bc[0:DK, :], ones_r[:], rr[:],
                                     start=True, stop=True)
                    if h2_p == 0:
                        nc.vector.tensor_tensor(
                            ot_sb[0:DK, hp_p, tc4 * 512:(tc4 + 1) * 512],
                            oa[:], bc[0:DK, :], mybir.AluOpType.mult)
                    else:
                        ob = o64.tile([DK, 512], BF16, tag="ob", name="ob")
                        nc.vector.tensor_tensor(
                            ob[:], oa[:], bc[0:DK, :], mybir.AluOpType.mult)
                        nc.sync.dma_start(
                            ot_sb[DK:P, hp_p, tc4 * 512:(tc4 + 1) * 512],
                            ob[:])

            def oproj_step(dt, tt, n2, alt=0):
                if alt % 2 == 0:
                    ps = ps_st.tile([P, 1024], F32, tag="ps", name="ps")
                else:
                    ps = ps_av.tile([P, 512], F32, tag="ps512", name="ps512")
                nc.tensor.matmul(
                    ps[:, 0:512],
                    ot_sb[:, dt, tt * P:(tt + 1) * P],
                    wo_sb[:, dt, n2 * 512:(n2 + 1) * 512],
                    start=True, stop=True)
                ysb = ysp.tile([P, 512], F32, tag="ysb", name="ysb")
                if alt % 4 < 2:
                    nc.vector.tensor_copy(ysb[:], ps[:, 0:512])
                else:
                    nc.scalar.copy(ysb[:], ps[:, 0:512])
                nc.sync.dma_start(
                    y[dt, tt * P:(tt + 1) * P, n2 * 512:(n2 + 1) * 512],
                    ysb[:])

            prev = None
            vps = None
            pjps = None
            for hp in range(2):
                for h2 in range(2):
                    h = hp * 2 + h2
                    r0 = h2 * DK
                    av_p = None
                    if prev is not None:
                        av_p = [ps_av.tile([P, 512], F32, tag="ps512",
                                           name="ps512") for _ in range(4)]
                        prev = (prev[0], prev[1], prev[2], av_p)
                    pt_cur = []
                    for st in range(TT):
                        ptt = ptp.tile([P, T], BF16, tag="pt", name="pt")
                        pt_cur.append(ptt)
                        for tch in range(2):
                            ps = ps_st.tile([P, 1024], F32, tag="ps", name="ps")
                            for u in range(2):
                                t0 = tch * 1024 + u * 512
                                nc.tensor.matmul(
                                    ps[:, u * 512:(u + 1) * 512],
                                    kt_sb[r0:r0 + DK, hp, st * P:(st + 1) * P],
                                    qt_sb[r0:r0 + DK, hp, t0:t0 + 512],
                                    start=True, stop=True)
                            nc.scalar.activation(
                                ptt[:, tch * 1024:(tch + 1) * 1024], ps[:],
                                mybir.ActivationFunctionType.Exp, scale=SCALE)
                        if h == 0:
                            # V projection and dt=1 QT/KT ride along head 0
                            if st % 2 == 0:
                                vps = ps_av.tile([P, 512], F32, tag="ps512",
                                                 name="ps512")
                            for e in range(ET):
                                nc.tensor.matmul(
                                    vps[:, (st % 2) * 256:(st % 2 + 1) * 256],
                                    xt_sb[:, e, st * P:(st + 1) * P],
                                    wv_sb[:, e, :],
                                    start=(e == 0), stop=(e == ET - 1))
                            if st % 2 == 1:
                                nc.vector.tensor_copy(
                                    v_sb[:, st - 1:st + 1, :, 0:DK],
                                    vps[:].rearrange(
                                        "p (s h d) -> p s h d", s=2, h=NH))
                            w_sb_p, dst_p, bcol_p, t0_p = proj1[st // 2]
                            if st % 2 == 0:
                                pjps = ps_av.tile([P, 512], F32, tag="ps512",
                                                  name="ps512")
                            for e in range(ET if st % 2 == 0 else 0):
                                pass
                            es = range(0, 4) if st % 2 == 0 else range(4, ET)
                            for e in es:
                                nc.tensor.matmul(
                                    pjps[:],
                                    w_sb_p[:, e, P:2 * P],
                                    xt_sb[:, e, t0_p:t0_p + 512],
                                    start=(e == 0), stop=(e == ET - 1))
                            if st % 2 == 1:
                                nc.vector.tensor_scalar_add(
                                    dst_p[:, 1, t0_p:t0_p + 512], pjps[:],
                                    bqk_sb[:, bcol_p + 1:bcol_p + 2])
                        if prev is not None:
                            av_step(prev, st)
                    if prev is not None:
                        av_drain(prev)
                    prev = (hp, h2, pt_cur, None)

            # epilogue: AV of the last head + dt0 output projection
            av_p = [ps_av.tile([P, 512], F32, tag="ps512", name="ps512")
                    for _ in range(4)]
            prev = (prev[0], prev[1], prev[2], av_p)
            od = [(tt, n2) for tt in range(TT) for n2 in range(2)]
            k = 0
            for st in range(TT):
                av_step(prev, st)
                for tt, n2 in od[st * 2 - 8:(st + 1) * 2 - 8] if st >= 4 else []:
                    oproj_step(0, tt, n2, k); k += 1
            av_drain(prev)
            for tt, n2 in od[24:]:
                oproj_step(0, tt, n2, k); k += 1
            for tt, n2 in od:
                oproj_step(1, tt, n2, k); k += 1
    nc.compile()
    return nc


_NC_CACHE = None


def _get_nc():
    global _NC_CACHE
    if _NC_CACHE is None:
        _NC_CACHE = build_nc()
    return _NC_CACHE


def _prep_inputs(x, Wq, bq, Wk, bk, Wv, bv, Wo, bo):
    bf = ml_dtypes.bfloat16
    in_maps = []
    for c in range(N_CORES):
        b, hg = c // 4, c % 4
        sl = slice(hg * DC, (hg + 1) * DC)
        bqk = np.empty((P, 4), np.float32)
        bqk[:, 0] = bq[sl][0:P]
        bqk[:, 1] = bq[sl][P:DC]
        bqk[:, 2] = bk[sl][0:P]
        bqk[:, 3] = bk[sl][P:DC]
        in_maps.append({
            "xT": np.ascontiguousarray(x[b].T).astype(bf),
            "wq": np.ascontiguousarray(Wq[sl, :].T).astype(bf),
            "wk": np.ascontiguousarray(Wk[sl, :].T).astype(bf),
            "wv": np.ascontiguousarray(Wv[sl, :].T).astype(bf),
            "wo": np.ascontiguousarray(Wo[:, sl].T).astype(bf),
            "bqk": bqk,
            "onec": np.ones((P, TT * NH), bf),
            "onesr": np.ones((1, DK), np.float32),
        })
    return in_maps


def _gather(results, Wo, bv, bo):
    bias = bo.astype(np.float64) + bv.astype(np.float64) @ Wo.T.astype(np.float64)
    out = np.empty((2, T, DM), np.float32)
    for b in range(2):
        acc = np.zeros((T, DM), np.float64)
        for hg in range(4):
            yv = results[b * 4 + hg]["y"]
            acc += yv[0]
            acc += yv[1]
        out[b] = (acc + bias).astype(np.float32)
    return out


def kernel(x, Wq, bq, Wk, bk, Wv, bv, Wo, bo, _trace=False, _res_box=None):
    x = np.asarray(x, np.float32)
    Wq, bq = np.asarray(Wq, np.float32), np.asarray(bq, np.float32)
    Wk, bk = np.asarray(Wk, np.float32), np.asarray(bk, np.float32)
    Wv, bv = np.asarray(Wv, np.float32), np.asarray(bv, np.float32)
    Wo, bo = np.asarray(Wo, np.float32), np.asarray(bo, np.float32)

    nc = _get_nc()
    in_maps = _prep_inputs(x, Wq, bq, Wk, bk, Wv, bv, Wo, bo)
    res = run_bass_kernel_spmd(nc, in_maps, core_ids=list(range(N_CORES)),
                               trace=_trace)
    if _res_box is not None:
        _res_box.append(res)
    return _gather(res.results, Wo, bv, bo)


# revision 29
# speedup vs baseline: 1.0900x; 1.0900x over previous
"""Multi-head attention (B=2, T=2048, D=1024, H=16, d_k=64) on 8 trn2 cores.

Sharding: tensor-parallel over heads (4 TP groups of 4 heads) x data-parallel
over batch (2). Core c handles batch (c // 4) and heads [4*(c%4), 4*(c%4)+4).

Per-core device program (all matmuls bf16 inputs, fp32 PSUM accumulate):
  QT/KT = W @ X^T            [256, 2048] (d-major layout, + bias via DVE)
  V     = X @ Wv^T           [2048, 256] (s-major, ones column appended)
  ST    = K^T-slices^T Q^T   scores transposed [s, t]; softmax runs over the
                             partition dim implicitly:
  P     = exp(ST / 32)       (ACT, no max-subtraction: |ST/32| < ~1)
  O'/r  = [V|1]^T P          one matmul gives both the unnormalized output and
                             the softmax denominator r (row 64)
  O     = O' * (1/r)         (DVE, gpsimd partition_broadcast for 1/r)
  Y     = O^T @ Wo-slice     [2048, 1024] partial, summed across TP on host.

Host folds the V bias and output bias: attn rows sum to 1, so
out += bo + bv @ Wo^T once per batch after the TP reduction.
"""

import numpy as np
import ml_dtypes

import concourse.bass as bass
from concourse import bacc
import concourse.mybir as mybir
import concourse.tile as tile
from concourse.bass_utils import run_bass_kernel_spmd

# ---------------------------------------------------------------------------

P = 128
T = 2048          # sequence length
DM = 1024         # d_model
DC = 256          # per-core head dims (4 heads x 64)
NH = 4            # heads per core
DK = 64
ET = DM // P      # 8 contraction tiles
TT = T // P       # 16 s/t tiles
N_CORES = 8
BF16 = mybir.dt.bfloat16
F32 = mybir.dt.float32
F32R = mybir.dt.float32r
SCALE = 1.0 / 32.0  # 1/sqrt(d_model)

_ts = bass.ts


def build_nc():
    nc = bacc.Bacc("TRN2", target_bir_lowering=False, debug=False)
    xT = nc.dram_tensor("xT", (DM, T), BF16, kind="ExternalInput")
    wq = nc.dram_tensor("wq", (DM, DC), BF16, kind="ExternalInput")
    wk = nc.dram_tensor("wk", (DM, DC), BF16, kind="ExternalInput")
    wv = nc.dram_tensor("wv", (DM, DC), BF16, kind="ExternalInput")
    wo = nc.dram_tensor("wo", (DC, DM), BF16, kind="ExternalInput")
    bqk = nc.dram_tensor("bqk", (P, 4), F32, kind="ExternalInput")
    onec = nc.dram_tensor("onec", (P, TT * NH), BF16, kind="ExternalInput")
    onesr = nc.dram_tensor("onesr", (1, DK), F32R, kind="ExternalInput")
    y = nc.dram_tensor("y", (T, DM), F32, kind="ExternalOutput")

    with tile.TileContext(nc) as tc:
        from contextlib import ExitStack
        with ExitStack() as ctx:
            cst = ctx.enter_context(tc.tile_pool(name="cst", bufs=1))
            wq_sb = cst.tile([P, ET, DC], BF16, tag="wq")
            wk_sb = cst.tile([P, ET, DC], BF16, tag="wk")
            wv_sb = cst.tile([P, ET, DC], BF16, tag="wv")
            wo_sb = cst.tile([P, 2, DM], BF16, tag="wo")
            bqk_sb = cst.tile([P, 4], F32, tag="bqk")
            ones_r = cst.tile([1, DK], F32R, tag="ones_r")
            xt_sb = cst.tile([P, ET, T], BF16, tag="xt")
            qt_sb = cst.tile([P, 2, T], BF16, tag="qt")
            kt_sb = cst.tile([P, 2, T], BF16, tag="kt")
            v_sb = cst.tile([P, TT, NH, DK + 1], BF16, tag="v")
            ot_sb = cst.tile([P, 2, T], BF16, tag="ot")

            nc.sync.dma_start(wq_sb[:], wq[:].rearrange("(e p) d -> p e d", p=P))
            nc.sync.dma_start(wk_sb[:], wk[:].rearrange("(e p) d -> p e d", p=P))
            nc.sync.dma_start(bqk_sb[:], bqk[:])
            xr = xT[:].rearrange("(e p) t -> p e t", p=P)
            for e in range(ET):
                nc.sync.dma_start(xt_sb[:, e, :], xr[:, e, :])
            nc.sync.dma_start(wv_sb[:], wv[:].rearrange("(e p) d -> p e d", p=P))
            nc.sync.dma_start(wo_sb[:], wo[:].rearrange("(g p) f -> p g f", p=P))
            nc.sync.dma_start(ones_r[:], onesr[:])
            nc.sync.dma_start(
                v_sb[:, :, :, DK:DK + 1],
                onec[:].rearrange("p (s h) -> p s h", s=TT, h=NH).unsqueeze(-1))

            ps_st = ctx.enter_context(
                tc.tile_pool(name="ps_st", bufs=2, space="PSUM"))
            ps_av = ctx.enter_context(
                tc.tile_pool(name="ps_av", bufs=4, space="PSUM"))
            ptp = ctx.enter_context(tc.tile_pool(name="ptp", bufs=22))
            rsp = ctx.enter_context(tc.tile_pool(name="rsp", bufs=2))
            rrp = ctx.enter_context(tc.tile_pool(name="rrp", bufs=4))
            o64 = ctx.enter_context(tc.tile_pool(name="o64", bufs=4))
            ysp = ctx.enter_context(tc.tile_pool(name="ysp", bufs=3))

            # ---- QT / KT dt=0 projections (pair-0 inputs) ------------------
            for w_sb, dst, bcol in ((wq_sb, qt_sb, 0), (wk_sb, kt_sb, 2)):
                for tcp in range(2):
                    ps = ps_st.tile([P, 1024], F32, tag="ps", name="ps")
                    for u in range(2):
                        t0 = tcp * 1024 + u * 512
                        for e in range(ET):
                            nc.tensor.matmul(
                                ps[:, u * 512:(u + 1) * 512],
                                w_sb[:, e, 0:P],
                                xt_sb[:, e, t0:t0 + 512],
                                start=(e == 0), stop=(e == ET - 1))
                    nc.vector.tensor_scalar_add(
                        dst[:, 0, tcp * 1024:(tcp + 1) * 1024], ps[:],
                        bqk_sb[:, bcol:bcol + 1])

            # dt=1 projection work, fed into head 0's stream as PE filler
            proj1 = [(w_sb, dst, bcol, t0)
                     for w_sb, dst, bcol in ((wq_sb, qt_sb, 0), (wk_sb, kt_sb, 2))
                     for t0 in (0, 512, 1024, 1536)]

            def av_step(prev_state, st):
                hp_p, h2_p, pt_p, av_p = prev_state
                hcol = hp_p * 2 + h2_p
                for tc4 in range(4):
                    nc.tensor.matmul(
                        av_p[tc4][0:DK + 1, :],
                        v_sb[:, st, hcol, :],
                        pt_p[st][:, tc4 * 512:(tc4 + 1) * 512],
                        start=(st == 0), stop=(st == TT - 1))

            def av_drain(prev_state):
                hp_p, h2_p, pt_p, av_p = prev_state
                oas, rrs, bcs = [], [], []
                for tc4 in range(4):
                    av = av_p[tc4]
                    oa = o64.tile([DK, 512], BF16, tag="oa", name="oa")
                    nc.vector.tensor_copy(oa[:], av[0:DK, :])
                    rs = rsp.tile([DK + 1, 512], F32, tag="rs", name="rs")
                    nc.vector.tensor_copy(rs[DK:DK + 1, :], av[DK:DK + 1, :])
                    rs0 = rsp.tile([1, 512], F32, tag="rs0", name="rs0")
                    nc.sync.dma_start(rs0[0:1, :], rs[DK:DK + 1, :])
                    rf = rsp.tile([1, 512], F32, tag="rf", name="rf")
                    nc.vector.reciprocal_approx_fast(rf[:], rs0[:])
                    rr = rrp.tile([1, 512], F32R, tag="rr", name="rr")
                    nc.vector.tensor_copy(rr[:], rf[:])
                    oas.append(oa); rrs.append(rr)
                for tc4 in range(4):
                    bc = ps_av.tile([P, 512], F32, tag="ps512", name="ps512")
                    nc.tensor.matmul(bc[0:DK, :], ones_r[:], rrs[tc4][:],
                                     start=True, stop=True)
                    bcs.append(bc)
                for tc4 in range(4):
                    if h2_p == 0:
                        nc.vector.tensor_tensor(
                            ot_sb[0:DK, hp_p, tc4 * 512:(tc4 + 1) * 512],
                            oas[tc4][:], bcs[tc4][0:DK, :],
                            mybir.AluOpType.mult)
                    else:
                        ob = o64.tile([DK, 512], BF16, tag="ob", name="ob")
                        nc.vector.tensor_tensor(
                            ob[:], oas[tc4][:], bcs[tc4][0:DK, :],
                            mybir.AluOpType.mult)
                        nc.sync.dma_start(
                            ot_sb[DK:P, hp_p, tc4 * 512:(tc4 + 1) * 512],
                            ob[:])

            def oproj_step(tt, n2, alt=0):
                if alt % 2 == 0:
                    ps = ps_st.tile([P, 1024], F32, tag="ps", name="ps")
                else:
                    ps = ps_av.tile([P, 512], F32, tag="ps512", name="ps512")
                for dt in range(2):
                    nc.tensor.matmul(
                        ps[:, 0:512],
                        ot_sb[:, dt, tt * P:(tt + 1) * P],
                        wo_sb[:, dt, n2 * 512:(n2 + 1) * 512],
                        start=(dt == 0), stop=(dt == 1))
                ysb = ysp.tile([P, 512], F32, tag="ysb", name="ysb")
                nc.vector.tensor_copy(ysb[:], ps[:, 0:512])
                nc.sync.dma_start(
                    y[tt * P:(tt + 1) * P, n2 * 512:(n2 + 1) * 512], ysb[:])

            prev = None
            vps = None
            pjps = None
            for hp in range(2):
                for h2 in range(2):
                    h = hp * 2 + h2
                    r0 = h2 * DK
                    av_p = None
                    if prev is not None:
                        av_p = [ps_av.tile([P, 512], F32, tag="ps512",
                                           name="ps512") for _ in range(4)]
                        prev = (prev[0], prev[1], prev[2], av_p)
                    pt_cur = []
                    for st in range(TT):
                        ptt = ptp.tile([P, T], BF16, tag="pt", name="pt")
                        pt_cur.append(ptt)
                        for tch in range(2):
                            ps = ps_st.tile([P, 1024], F32, tag="ps", name="ps")
                            for u in range(2):
                                t0 = tch * 1024 + u * 512
                                nc.tensor.matmul(
                                    ps[:, u * 512:(u + 1) * 512],
                                    kt_sb[r0:r0 + DK, hp, st * P:(st + 1) * P],
                                    qt_sb[r0:r0 + DK, hp, t0:t0 + 512],
                                    start=True, stop=True)
                            nc.scalar.activation(
                                ptt[:, tch * 1024:(tch + 1) * 1024], ps[:],
                                mybir.ActivationFunctionType.Exp, scale=SCALE)
                        if h == 0:
                            # V projection and dt=1 QT/KT ride along head 0
                            if st % 2 == 0:
                                vps = ps_av.tile([P, 512], F32, tag="ps512",
                                                 name="ps512")
                            for e in range(ET):
                                nc.tensor.matmul(
                                    vps[:, (st % 2) * 256:(st % 2 + 1) * 256],
                                    xt_sb[:, e, st * P:(st + 1) * P],
                                    wv_sb[:, e, :],
                                    start=(e == 0), stop=(e == ET - 1))
                            if st % 2 == 1:
                                nc.vector.tensor_copy(
                                    v_sb[:, st - 1:st + 1, :, 0:DK],
                                    vps[:].rearrange(
                                        "p (s h d) -> p s h d", s=2, h=NH))
                            w_sb_p, dst_p, bcol_p, t0_p = proj1[st // 2]
                            if st % 2 == 0:
                                pjps = ps_av.tile([P, 512], F32, tag="ps512",
                                                  name="ps512")
                            for e in range(ET if st % 2 == 0 else 0):
                                pass
                            es = range(0, 4) if st % 2 == 0 else range(4, ET)
                            for e in es:
                                nc.tensor.matmul(
                                    pjps[:],
                                    w_sb_p[:, e, P:2 * P],
                                    xt_sb[:, e, t0_p:t0_p + 512],
                                    start=(e == 0), stop=(e == ET - 1))
                            if st % 2 == 1:
                                nc.vector.tensor_scalar_add(
                                    dst_p[:, 1, t0_p:t0_p + 512], pjps[:],
                                    bqk_sb[:, bcol_p + 1:bcol_p + 2])
                        if prev is not None:
                            av_step(prev, st)
                    if prev is not None:
                        av_drain(prev)
                    prev = (hp, h2, pt_cur, None)

            # epilogue: AV of the last head + dt0 output projection
            av_p = [ps_av.tile([P, 512], F32, tag="ps512", name="ps512")
                    for _ in range(4)]
            prev = (prev[0], prev[1], prev[2], av_p)
            for st in range(TT):
                av_step(prev, st)
            av_drain(prev)
            k = 0
            for tt in range(TT):
                for n2 in range(2):
                    oproj_step(tt, n2, k); k += 1
    nc.compile()
    return nc


_NC_CACHE = None


def _get_nc():
    global _NC_CACHE
    if _NC_CACHE is None:
        _NC_CACHE = build_nc()
    return _NC_CACHE


def _prep_inputs(x, Wq, bq, Wk, bk, Wv, bv, Wo, bo):
    bf = ml_dtypes.bfloat16
    in_maps = []
    for c in range(N_CORES):
        b, hg = c // 4, c % 4
        sl = slice(hg * DC, (hg + 1) * DC)
        bqk = np.empty((P, 4), np.float32)
        bqk[:, 0] = bq[sl][0:P]
        bqk[:, 1] = bq[sl][P:DC]
        bqk[:, 2] = bk[sl][0:P]
        bqk[:, 3] = bk[sl][P:DC]
        in_maps.append({
            "xT": np.ascontiguousarray(x[b].T).astype(bf),
            "wq": np.ascontiguousarray(Wq[sl, :].T).astype(bf),
            "wk": np.ascontiguousarray(Wk[sl, :].T).astype(bf),
            "wv": np.ascontiguousarray(Wv[sl, :].T).astype(bf),
            "wo": np.ascontiguousarray(Wo[:, sl].T).astype(bf),
            "bqk": bqk,
            "onec": np.ones((P, TT * NH), bf),
            "onesr": np.ones((1, DK), np.float32),
        })
    return in_maps


def _gather(results, Wo, bv, bo):
    bias = bo.astype(np.float64) + bv.astype(np.float64) @ Wo.T.astype(np.float64)
    out = np.empty((2, T, DM), np.float32)
    for b in range(2):
        acc = np.zeros((T, DM), np.float64)
        for hg in range(4):
            acc += results[b * 4 + hg]["y"]
        out[b] = (acc + bias).astype(np.float32)
    return out


def kernel(x, Wq, bq, Wk, bk, Wv, bv, Wo, bo, _trace=False, _res_box=None):
    x = np.asarray(x, np.float32)
    Wq, bq = np.asarray(Wq, np.float32), np.asarray(bq, np.float32)
    Wk, bk = np.asarray(Wk, np.float32), np.asarray(bk, np.float32)
    Wv, bv = np.asarray(Wv, np.float32), np.asarray(bv, np.float32)
    Wo, bo = np.asarray(Wo, np.float32), np.asarray(bo, np.float32)

    nc = _get_nc()
    in_maps = _prep_inputs(x, Wq, bq, Wk, bk, Wv, bv, Wo, bo)
    res = run_bass_kernel_spmd(nc, in_maps, core_ids=list(range(N_CORES)),
                               trace=_trace)
    if _res_box is not None:
        _res_box.append(res)
    return _gather(res.results, Wo, bv, bo)
